# revision 9
# baseline (speedup 1.0000x reference)
"""Multi-head attention (B*H=64, S=2048, D=64) on 8 Trainium2 cores.

Sharding: 64 heads -> 8 per core (head-parallel, no communication).

Per-core kernel (heads processed in pairs A/B stacked on SBUF partition
halves 0:64 / 64:128):
  - prep (per pair, interleaved into the previous pair's main loop via a
    deferred-closure queue): q/k are DMA'd with a 32x32-block-permuted
    access pattern, DVE 32x32 block-transposed, and GPSIMD-cast to f16
    stacked Q^T/K^T [128(dA|dB), 2048].  V is loaded naturally and
    GPSIMD-cast to bf16 with an appended ones column (so the PV matmul
    also produces the softmax denominator).
  - main loop per pair x (q-chunk 512) x (16 k-tiles of 128):
      S^T[k,q] = K Q^T   -- two row-packed f16 matmuls
                            (tile_position (0,0)/(64,0)), concurrent on PE
      P^T = exp(S^T)     -- bf16; exact exp on ACT for ~8.5/16 k-tiles,
                            Schraudolph int16 bit-trick on DVE for ~7.5
      O^T[d+1,q] += V_aug^T P^T -- bf16 matmul, PSUM-accumulated;
                            row 64 accumulates the denominator Z
    QK^T runs 3 k-tiles ahead (spsum bufs=3) so the PSUM-slot-reuse
    dependency exp(kt) -> QK(kt+3) never sits on the critical path.
  - epilogue per (pair, g): the PSUM->SBUF copies (ACT+DVE) and PE
    transposes are emitted at the START of the next chunk, AFTER its
    first three QK pairs, so the PE FIFO never head-blocks at chunk
    boundaries (keeps HAM warm).  The tail (ep->SBUF copy on ACT, 1/Z
    reciprocal on DVE, scale on GPSIMD, DMA out) is deferred into the
    next chunk's kt loop via a closure queue, one op per kt.
"""

import os

import numpy as np

import concourse.bass as bass
import concourse.mybir as mybir
import concourse.tile as tile
from concourse import bacc
from concourse.bass_utils import run_bass_kernel_spmd
from concourse.masks import make_identity

B, S, D = 64, 2048, 64
NCORES = 8
H = B // NCORES  # heads per core
P = 128  # partitions
KT = S // P  # 16 k-tiles
QC = 512  # q-chunk
NQC = S // QC  # 4 q-chunks
NPAIR = H // 2  # head pairs per core

F32 = mybir.dt.float32
BF16 = mybir.dt.bfloat16
I16 = mybir.dt.int16
F16 = mybir.dt.float16

# Base number of k-tiles (of 16) whose exp runs on DVE via the Schraudolph
# bit trick; alternating chunks use DVE_BASE / DVE_BASE+1 tiles.
DVE_BASE = int(os.environ.get("BASS_ATTN_DVE_BASE", "7"))


def _dve_set(size):
    return {round((i + 0.5) * KT / size) % KT for i in range(size)}


# Schraudolph constants for bf16 exp via int16 bit pattern:
#   i = round_int16(x * 2^7/ln2 + b);  exp(x) ~= bitcast_bf16(i)
_SCH_A = float(128.0 / np.log(2.0))
_SCH_B = float(os.environ.get("BASS_ATTN_SCH_B", "16250.5"))


def build_attention_nc() -> bass.Bass:
    nc = bacc.Bacc()
    q_d = nc.declare_dram_parameter("q", [H, S, D], F32, isOutput=False)
    k_d = nc.declare_dram_parameter("k", [H, S, D], F32, isOutput=False)
    v_d = nc.declare_dram_parameter("v", [H, S, D], F32, isOutput=False)
    o_d = nc.declare_dram_parameter("out", [H, S, D], F32, isOutput=True)

    q_bp = q_d.rearrange("h (b i) (a j) -> h a i b j", i=32, j=32)
    k_bp = k_d.rearrange("h (b i) (a j) -> h a i b j", i=32, j=32)
    v_v = v_d.rearrange("h (t p) d -> h p t d", p=P)
    o_v = o_d.rearrange("h (g c p) d -> h p g c d", c=4, p=P)

    with tile.TileContext(nc) as tc:
        with (
            tc.tile_pool(name="consts", bufs=1) as consts,
            tc.tile_pool(name="stage", bufs=4) as stage,
            tc.tile_pool(name="qk_t", bufs=16) as qkt_pool,
            tc.tile_pool(name="vpool", bufs=4) as vpool,
            tc.tile_pool(name="ppool", bufs=6) as ppool,
            tc.tile_pool(name="osb", bufs=4) as osb_pool,
            tc.tile_pool(name="epsb", bufs=4) as epsb_pool,
            tc.tile_pool(name="outsb", bufs=4) as outsb_pool,
            tc.tile_pool(name="rz", bufs=4) as rz_pool,
            tc.tile_pool(name="spsum", bufs=3, space="PSUM") as spsum,
            tc.tile_pool(name="oep", bufs=2, space="PSUM") as opsum,
        ):
            ident = consts.tile([D + 1, D + 1], F32)
            make_identity(nc, ident[:])
            ones16 = consts.tile([P, KT], F32)
            nc.vector.memset(ones16[:], 1.0)

            # Deferred-work queue (prep of next pair + epilogue tails),
            # drained one closure per kt so per-engine FIFOs interleave
            # boundary work with steady-state work.
            pending = []

            def drain(n=1):
                for _ in range(min(n, len(pending))):
                    pending.pop(0)()

            def prep_closures(pair):
                h_a, h_b = 2 * pair, 2 * pair + 1
                qkt = {"q": [None] * 4, "k": [None] * 4}
                v_aug = {}
                out = []
                for fc in range(4):
                    for name, src in (("k", k_bp), ("q", q_bp)):

                        def go(fc=fc, name=name, src=src):
                            st = stage.tile([P, QC], F32, tag="stage")
                            for hh, pb in ((h_a, 0), (h_b, 2)):
                                for a in range(2):
                                    c = pb + a
                                    nc.sync.dma_start(
                                        out=st[32 * c : 32 * c + 32, :].rearrange(
                                            "i (b j) -> i b j", j=32
                                        ),
                                        in_=src[hh, a, :, 16 * fc : 16 * fc + 16, :],
                                    )
                            st2 = stage.tile([P, QC], F32, tag="stage2")
                            nc.vector.transpose(st2[:], st[:])
                            ch = qkt_pool.tile([P, QC], F16, tag="qkT")
                            nc.gpsimd.tensor_copy(ch[:], st2[:])
                            qkt[name][fc] = ch

                        out.append(go)
                for hh, part in ((h_a, 0), (h_b, 1)):

                    def gov(hh=hh, part=part):
                        vst = stage.tile([P, KT, D], F32, tag="vstage")
                        nc.sync.dma_start(out=vst[:], in_=v_v[hh])
                        va = vpool.tile([P, KT, D + 1], BF16, tag="v")
                        nc.gpsimd.tensor_copy(va[:, :, 0:D], vst[:])
                        nc.gpsimd.tensor_copy(va[:, :, D], ones16[:])
                        v_aug[part] = va

                    out.append(gov)
                return out, qkt, v_aug

            def epilogue_core(o_ps):
                """Inline at next-chunk start (after its QK prefetch):
                PSUM->SBUF copies (ACT part0 / DVE part1), PE transposes,
                and ep->SBUF copies.  After this, the o_ps and ep PSUM
                slots have all readers emitted, so the 2-deep oep pool
                rotation stays legal."""
                epcs = {}
                osbs = {}
                for part in (0, 1):
                    o_sb = osb_pool.tile([D + 1, QC], F32, tag="ot")
                    if part == 0:
                        nc.scalar.copy(o_sb[:], o_ps[part][:])
                    else:
                        nc.vector.tensor_copy(o_sb[:], o_ps[part][:])
                    osbs[part] = o_sb
                for part in (0, 1):
                    ep = opsum.tile([P, 4, D + 1], F32, tag="oep")
                    for c in range(4):
                        nc.tensor.transpose(
                            ep[:, c, :],
                            osbs[part][:, c * P : (c + 1) * P],
                            ident[:],
                        )
                    epc = epsb_pool.tile([P, 4, D + 1], F32, tag="eps")
                    if part == 0:
                        nc.scalar.copy(epc[:], ep[:])
                    else:
                        nc.vector.tensor_copy(epc[:], ep[:])
                    epcs[part] = epc
                return epcs

            def epilogue_tail_closures(pair, g, epcs):
                """Deferred: 1/Z recip (DVE), scale (GPSIMD), DMA out.
                All-SBUF, so ordering vs PSUM slot rotation is free."""
                h_a, h_b = 2 * pair, 2 * pair + 1
                out = []
                for part, hh in ((0, h_a), (1, h_b)):

                    def e_fin(part=part, hh=hh):
                        epc = epcs[part]
                        rz = rz_pool.tile([P, 4], F32, tag="rz")
                        nc.vector.reciprocal(rz[:], epc[:, :, D])
                        out_sb = outsb_pool.tile([P, 4, D], F32, tag="out")
                        for c in range(4):
                            nc.gpsimd.tensor_scalar(
                                out=out_sb[:, c, :],
                                in0=epc[:, c, 0:D],
                                scalar1=rz[:, c : c + 1],
                                scalar2=None,
                                op0=mybir.AluOpType.mult,
                            )
                        nc.sync.dma_start(out=o_v[hh, :, g, :, :], in_=out_sb[:])

                    out.append(e_fin)
                return out

            prep0, qkt, v_aug = prep_closures(0)
            for c in prep0:
                c()
            nxt = {}
            prev = None  # (pair, g, o_ps) of the chunk awaiting epilogue

            chunk_idx = 0
            for pair in range(NPAIR):
                if pair > 0:
                    qkt, v_aug = nxt[pair]
                    while any(t is None for t in qkt["q"] + qkt["k"]) or len(
                        v_aug
                    ) < 2:
                        pending.pop(0)()
                kT = qkt["k"]
                qT = qkt["q"]

                for g in range(NQC):

                    def emit_qkt(kt):
                        s_ps = spsum.tile([P, 2, QC], F32, tag="s")
                        k_ch = kT[kt // 4]
                        k_sl = slice((kt % 4) * P, (kt % 4 + 1) * P)
                        for part, base in ((0, 0), (1, 64)):
                            nc.tensor.matmul(
                                s_ps[:, part, :],
                                k_ch[base : base + 64, k_sl],
                                qT[g][base : base + 64, :],
                                tile_position=(base, 0),
                            )
                        return s_ps

                    # QK prefetch for this chunk FIRST, so the PE FIFO has
                    # work before the previous chunk's epilogue transposes.
                    s_tiles = {i: emit_qkt(i) for i in range(3)}

                    # previous chunk's epilogue core (frees its o_ps slots
                    # before we allocate ours from the 2-deep pool)
                    if prev is not None:
                        p_pair, p_g, p_ops = prev
                        eps = epilogue_core(p_ops)
                        pending.extend(epilogue_tail_closures(p_pair, p_g, eps))

                    o_ps_a = opsum.tile([D + 1, QC], F32, tag="oep")
                    o_ps_b = opsum.tile([D + 1, QC], F32, tag="oep")
                    o_ps = {0: o_ps_a, 1: o_ps_b}

                    dve_kts = _dve_set(DVE_BASE + (chunk_idx & 1))
                    for kt in range(KT):
                        s_ps = s_tiles.pop(kt)
                        p_sb = ppool.tile([P, 2, QC], BF16, tag="p")
                        if kt in dve_kts:
                            nc.vector.tensor_scalar(
                                out=p_sb[:].bitcast(I16),
                                in0=s_ps[:],
                                scalar1=_SCH_A,
                                scalar2=_SCH_B,
                                op0=mybir.AluOpType.mult,
                                op1=mybir.AluOpType.add,
                            )
                        else:
                            nc.scalar.activation(
                                p_sb[:], s_ps[:], mybir.ActivationFunctionType.Exp
                            )
                        for part in (0, 1):
                            nc.tensor.matmul(
                                o_ps[part][:],
                                v_aug[part][:, kt, :],
                                p_sb[:, part, :],
                                start=(kt == 0),
                                stop=(kt == KT - 1),
                            )
                        if kt + 3 < KT:
                            s_tiles[kt + 3] = emit_qkt(kt + 3)
                        drain(1)

                    prev = (pair, g, o_ps)
                    if g == 1 and pair + 1 < NPAIR:
                        pc, qkt_n, v_n = prep_closures(pair + 1)
                        pending.extend(pc)
                        nxt[pair + 1] = (qkt_n, v_n)
                    chunk_idx += 1

            # final chunk's epilogue + leftover deferred work
            p_pair, p_g, p_ops = prev
            eps = epilogue_core(p_ops)
            pending.extend(epilogue_tail_closures(p_pair, p_g, eps))
            drain(len(pending))
    nc.finalize()
    return nc


_NC_CACHE = None


def _get_nc():
    global _NC_CACHE
    if _NC_CACHE is None:
        _NC_CACHE = build_attention_nc()
    return _NC_CACHE


def kernel(q: np.ndarray, k: np.ndarray, v: np.ndarray) -> np.ndarray:
    q = np.asarray(q, dtype=np.float32)
    k = np.asarray(k, dtype=np.float32)
    v = np.asarray(v, dtype=np.float32)
    nc = _get_nc()
    in_maps = [
        {
            "q": np.ascontiguousarray(q[c * H : (c + 1) * H]),
            "k": np.ascontiguousarray(k[c * H : (c + 1) * H]),
            "v": np.ascontiguousarray(v[c * H : (c + 1) * H]),
        }
        for c in range(NCORES)
    ]
    res = run_bass_kernel_spmd(nc, in_maps, list(range(NCORES)))
    return np.concatenate([res.results[c]["out"] for c in range(NCORES)], axis=0)


# revision 11
# speedup vs baseline: 1.1533x; 1.1533x over previous
"""Multi-head attention (B*H=64, S=2048, D=64) on 8 Trainium2 cores.

Sharding: 64 heads -> 8 per core (head-parallel, no communication).

Per-core kernel (heads processed in pairs A/B stacked on SBUF partition
halves 0:64 / 64:128):
  - prep (per pair, interleaved into the previous pair's main loop via a
    deferred-closure queue): q/k are DMA'd with a 32x32-block-permuted
    access pattern, DVE 32x32 block-transposed, and GPSIMD-cast to f16
    stacked Q^T/K^T [128(dA|dB), 2048].  V is loaded naturally and
    GPSIMD-cast to bf16 with an appended ones column (so the PV matmul
    also produces the softmax denominator).
  - main loop per pair x (q-chunk 512) x (16 k-tiles of 128):
      S^T[k,q] = K Q^T   -- two row-packed f16 matmuls
                            (tile_position (0,0)/(64,0)), concurrent on PE
      P^T = exp(S^T)     -- bf16; exact exp on ACT for ~8.5/16 k-tiles,
                            Schraudolph int16 bit-trick on DVE for ~7.5
      O^T[d+1,q] += V_aug^T P^T -- bf16 matmul, PSUM-accumulated;
                            row 64 accumulates the denominator Z
    QK^T runs 3 k-tiles ahead (spsum bufs=3) so the PSUM-slot-reuse
    dependency exp(kt) -> QK(kt+3) never sits on the critical path.
  - epilogue per (pair, g): the PSUM->SBUF copies (ACT+DVE) and PE
    transposes are emitted at the START of the next chunk, AFTER its
    first three QK pairs, so the PE FIFO never head-blocks at chunk
    boundaries (keeps HAM warm).  The tail (ep->SBUF copy on ACT, 1/Z
    reciprocal on DVE, scale on GPSIMD, DMA out) is deferred into the
    next chunk's kt loop via a closure queue, one op per kt.
"""

import os

import numpy as np

import concourse.bass as bass
import concourse.mybir as mybir
import concourse.tile as tile
from concourse import bacc
from concourse.bass_utils import run_bass_kernel_spmd
from concourse.masks import make_identity

B, S, D = 64, 2048, 64
NCORES = 8
H = B // NCORES  # heads per core
P = 128  # partitions
KT = S // P  # 16 k-tiles
QC = 512  # q-chunk
NQC = S // QC  # 4 q-chunks
NPAIR = H // 2  # head pairs per core

F32 = mybir.dt.float32
BF16 = mybir.dt.bfloat16
I16 = mybir.dt.int16
F16 = mybir.dt.float16

# Base number of k-tiles (of 16) whose exp runs on DVE via the Schraudolph
# bit trick; alternating chunks use DVE_BASE / DVE_BASE+1 tiles.
DVE_BASE = int(os.environ.get("BASS_ATTN_DVE_BASE", "6"))


def _dve_set(size):
    return {round((i + 0.5) * KT / size) % KT for i in range(size)}


# Schraudolph constants for bf16 exp via int16 bit pattern:
#   i = round_int16(x * 2^7/ln2 + b);  exp(x) ~= bitcast_bf16(i)
_SCH_A = float(128.0 / np.log(2.0))
_SCH_B = float(os.environ.get("BASS_ATTN_SCH_B", "16250.5"))


def build_attention_nc() -> bass.Bass:
    nc = bacc.Bacc()
    q_d = nc.declare_dram_parameter("q", [H, S, D], F32, isOutput=False)
    k_d = nc.declare_dram_parameter("k", [H, S, D], F32, isOutput=False)
    v_d = nc.declare_dram_parameter("v", [H, S, D], F32, isOutput=False)
    o_d = nc.declare_dram_parameter("out", [H, S, D], F32, isOutput=True)

    q_bp = q_d.rearrange("h (b i) (a j) -> h a i b j", i=32, j=32)
    k_bp = k_d.rearrange("h (b i) (a j) -> h a i b j", i=32, j=32)
    v_v = v_d.rearrange("h (t p) d -> h p t d", p=P)
    o_v = o_d.rearrange("h (g c p) d -> h p g c d", c=4, p=P)

    with tile.TileContext(nc) as tc:
        with (
            tc.tile_pool(name="consts", bufs=1) as consts,
            tc.tile_pool(name="stage", bufs=4) as stage,
            tc.tile_pool(name="qk_t", bufs=16) as qkt_pool,
            tc.tile_pool(name="vpool", bufs=4) as vpool,
            tc.tile_pool(name="ppool", bufs=6) as ppool,
            tc.tile_pool(name="osb", bufs=4) as osb_pool,
            tc.tile_pool(name="epsb", bufs=4) as epsb_pool,
            tc.tile_pool(name="outsb", bufs=4) as outsb_pool,
            tc.tile_pool(name="rz", bufs=4) as rz_pool,
            tc.tile_pool(name="spsum", bufs=3, space="PSUM") as spsum,
            tc.tile_pool(name="oep", bufs=2, space="PSUM") as opsum,
        ):
            ident = consts.tile([D + 1, D + 1], F32)
            make_identity(nc, ident[:])
            ones16 = consts.tile([P, KT], F32)
            nc.vector.memset(ones16[:], 1.0)

            # Deferred-work queue (prep of next pair + epilogue tails),
            # drained one closure per kt so per-engine FIFOs interleave
            # boundary work with steady-state work.
            pending = []

            def drain(n=1):
                for _ in range(min(n, len(pending))):
                    pending.pop(0)()

            def prep_closures(pair):
                h_a, h_b = 2 * pair, 2 * pair + 1
                qkt = {"q": [None] * 4, "k": [None] * 4}
                v_aug = {}
                out = []
                for fc in range(4):
                    for name, src in (("k", k_bp), ("q", q_bp)):

                        def go(fc=fc, name=name, src=src):
                            st = stage.tile([P, QC], F32, tag="stage")
                            for hh, pb in ((h_a, 0), (h_b, 2)):
                                for a in range(2):
                                    c = pb + a
                                    nc.sync.dma_start(
                                        out=st[32 * c : 32 * c + 32, :].rearrange(
                                            "i (b j) -> i b j", j=32
                                        ),
                                        in_=src[hh, a, :, 16 * fc : 16 * fc + 16, :],
                                    )
                            st2 = stage.tile([P, QC], F32, tag="stage2")
                            nc.vector.transpose(st2[:], st[:])
                            ch = qkt_pool.tile([P, QC], F16, tag="qkT")
                            nc.vector.tensor_copy(ch[:], st2[:])
                            qkt[name][fc] = ch

                        out.append(go)
                for hh, part in ((h_a, 0), (h_b, 1)):

                    def gov(hh=hh, part=part):
                        vst = stage.tile([P, KT, D], F32, tag="vstage")
                        nc.sync.dma_start(out=vst[:], in_=v_v[hh])
                        va = vpool.tile([P, KT, D + 1], BF16, tag="v")
                        nc.vector.tensor_copy(va[:, :, 0:D], vst[:])
                        nc.vector.tensor_copy(va[:, :, D], ones16[:])
                        v_aug[part] = va

                    out.append(gov)
                return out, qkt, v_aug

            def epilogue_core(o_ps):
                """Inline at next-chunk start (after its QK prefetch):
                PSUM->SBUF copies (ACT part0 / DVE part1), PE transposes,
                and ep->SBUF copies.  All PSUM readers are emitted here,
                so the 2-deep oep slot rotation (o_a, o_b, ep_a, ep_b per
                chunk) stays legal; the deferred tail is SBUF-only."""
                epcs = {}
                osbs = {}
                for part in (0, 1):
                    o_sb = osb_pool.tile([D + 1, QC], F32, tag="ot")
                    if part == 0:
                        nc.scalar.copy(o_sb[:], o_ps[part][:])
                    else:
                        nc.vector.tensor_copy(o_sb[:], o_ps[part][:])
                    osbs[part] = o_sb
                for part in (0, 1):
                    ep = opsum.tile([P, 4, D + 1], F32, tag="oep")
                    for c in range(4):
                        nc.tensor.transpose(
                            ep[:, c, :],
                            osbs[part][:, c * P : (c + 1) * P],
                            ident[:],
                        )
                    epc = epsb_pool.tile([P, 4, D + 1], F32, tag="eps")
                    if part == 0:
                        nc.scalar.copy(epc[:], ep[:])
                    else:
                        nc.vector.tensor_copy(epc[:], ep[:])
                    epcs[part] = epc
                return epcs

            def epilogue_tail_closures(pair, g, epcs):
                """Deferred into the next chunk, one per kt: 1/Z recip +
                scale on DVE -- all SBUF reads, so tensor_scalar gets the
                2-port mode -- then DMA out."""
                h_a, h_b = 2 * pair, 2 * pair + 1
                out = []
                for part, hh in ((0, h_a), (1, h_b)):

                    def e_fin(part=part, hh=hh):
                        epc = epcs[part]
                        rz = rz_pool.tile([P, 4], F32, tag="rz")
                        nc.vector.reciprocal(rz[:], epc[:, :, D])
                        out_sb = outsb_pool.tile([P, 4, D], F32, tag="out")
                        for c in range(4):
                            nc.vector.tensor_scalar(
                                out=out_sb[:, c, :],
                                in0=epc[:, c, 0:D],
                                scalar1=rz[:, c : c + 1],
                                scalar2=None,
                                op0=mybir.AluOpType.mult,
                            )
                        nc.sync.dma_start(out=o_v[hh, :, g, :, :], in_=out_sb[:])

                    out.append(e_fin)
                return out

            prep0, qkt, v_aug = prep_closures(0)
            for c in prep0:
                c()
            nxt = {}
            prev = None  # (pair, g, o_ps) of the chunk awaiting epilogue

            chunk_idx = 0
            for pair in range(NPAIR):
                if pair > 0:
                    qkt, v_aug = nxt[pair]
                    while any(t is None for t in qkt["q"] + qkt["k"]) or len(
                        v_aug
                    ) < 2:
                        pending.pop(0)()
                kT = qkt["k"]
                qT = qkt["q"]

                for g in range(NQC):

                    def emit_qkt(kt):
                        s_ps = spsum.tile([P, 2, QC], F32, tag="s")
                        k_ch = kT[kt // 4]
                        k_sl = slice((kt % 4) * P, (kt % 4 + 1) * P)
                        for part, base in ((0, 0), (1, 64)):
                            nc.tensor.matmul(
                                s_ps[:, part, :],
                                k_ch[base : base + 64, k_sl],
                                qT[g][base : base + 64, :],
                                tile_position=(base, 0),
                            )
                        return s_ps

                    # QK prefetch for this chunk FIRST, so the PE FIFO has
                    # work before the previous chunk's epilogue transposes.
                    s_tiles = {i: emit_qkt(i) for i in range(3)}

                    # previous chunk's epilogue core (frees its o_ps slots
                    # before we allocate ours from the 2-deep pool)
                    if prev is not None:
                        p_pair, p_g, p_ops = prev
                        eps = epilogue_core(p_ops)
                        pending.extend(epilogue_tail_closures(p_pair, p_g, eps))

                    o_ps_a = opsum.tile([D + 1, QC], F32, tag="oep")
                    o_ps_b = opsum.tile([D + 1, QC], F32, tag="oep")
                    o_ps = {0: o_ps_a, 1: o_ps_b}

                    dve_kts = _dve_set(DVE_BASE)
                    for kt in range(KT):
                        s_ps = s_tiles.pop(kt)
                        p_sb = ppool.tile([P, 2, QC], BF16, tag="p")
                        if kt in dve_kts:
                            nc.vector.tensor_scalar(
                                out=p_sb[:].bitcast(I16),
                                in0=s_ps[:],
                                scalar1=_SCH_A,
                                scalar2=_SCH_B,
                                op0=mybir.AluOpType.mult,
                                op1=mybir.AluOpType.add,
                            )
                        else:
                            nc.scalar.activation(
                                p_sb[:], s_ps[:], mybir.ActivationFunctionType.Exp
                            )
                        for part in (0, 1):
                            nc.tensor.matmul(
                                o_ps[part][:],
                                v_aug[part][:, kt, :],
                                p_sb[:, part, :],
                                start=(kt == 0),
                                stop=(kt == KT - 1),
                            )
                        if kt + 3 < KT:
                            s_tiles[kt + 3] = emit_qkt(kt + 3)
                        drain(1)

                    prev = (pair, g, o_ps)
                    if g == 1 and pair + 1 < NPAIR:
                        pc, qkt_n, v_n = prep_closures(pair + 1)
                        pending.extend(pc)
                        nxt[pair + 1] = (qkt_n, v_n)
                    chunk_idx += 1

            # final chunk's epilogue + leftover deferred work
            p_pair, p_g, p_ops = prev
            eps = epilogue_core(p_ops)
            pending.extend(epilogue_tail_closures(p_pair, p_g, eps))
            drain(len(pending))
    nc.finalize()
    return nc


_NC_CACHE = None


def _get_nc():
    global _NC_CACHE
    if _NC_CACHE is None:
        _NC_CACHE = build_attention_nc()
    return _NC_CACHE


def kernel(q: np.ndarray, k: np.ndarray, v: np.ndarray) -> np.ndarray:
    q = np.asarray(q, dtype=np.float32)
    k = np.asarray(k, dtype=np.float32)
    v = np.asarray(v, dtype=np.float32)
    nc = _get_nc()
    in_maps = [
        {
            "q": np.ascontiguousarray(q[c * H : (c + 1) * H]),
            "k": np.ascontiguousarray(k[c * H : (c + 1) * H]),
            "v": np.ascontiguousarray(v[c * H : (c + 1) * H]),
        }
        for c in range(NCORES)
    ]
    res = run_bass_kernel_spmd(nc, in_maps, list(range(NCORES)))
    return np.concatenate([res.results[c]["out"] for c in range(NCORES)], axis=0)


# revision 12
# speedup vs baseline: 1.1665x; 1.0114x over previous
"""Multi-head attention (B*H=64, S=2048, D=64) on 8 Trainium2 cores.

Sharding: 64 heads -> 8 per core (head-parallel, no communication).

Per-core kernel (heads processed in pairs A/B stacked on SBUF partition
halves 0:64 / 64:128):
  - prep (per pair, interleaved into the previous pair's main loop via a
    deferred-closure queue): q/k are DMA'd with a 32x32-block-permuted
    access pattern, DVE 32x32 block-transposed, and GPSIMD-cast to f16
    stacked Q^T/K^T [128(dA|dB), 2048].  V is loaded naturally and
    GPSIMD-cast to bf16 with an appended ones column (so the PV matmul
    also produces the softmax denominator).
  - main loop per pair x (q-chunk 512) x (16 k-tiles of 128):
      S^T[k,q] = K Q^T   -- two row-packed f16 matmuls
                            (tile_position (0,0)/(64,0)), concurrent on PE
      P^T = exp(S^T)     -- bf16; exact exp on ACT for ~8.5/16 k-tiles,
                            Schraudolph int16 bit-trick on DVE for ~7.5
      O^T[d+1,q] += V_aug^T P^T -- bf16 matmul, PSUM-accumulated;
                            row 64 accumulates the denominator Z
    QK^T runs 3 k-tiles ahead (spsum bufs=3) so the PSUM-slot-reuse
    dependency exp(kt) -> QK(kt+3) never sits on the critical path.
  - epilogue per (pair, g): the PSUM->SBUF copies (ACT+DVE) and PE
    transposes are emitted at the START of the next chunk, AFTER its
    first three QK pairs, so the PE FIFO never head-blocks at chunk
    boundaries (keeps HAM warm).  The tail (ep->SBUF copy on ACT, 1/Z
    reciprocal on DVE, scale on GPSIMD, DMA out) is deferred into the
    next chunk's kt loop via a closure queue, one op per kt.
"""

import os

import numpy as np

import concourse.bass as bass
import concourse.mybir as mybir
import concourse.tile as tile
from concourse import bacc
from concourse.bass_utils import run_bass_kernel_spmd
from concourse.masks import make_identity

B, S, D = 64, 2048, 64
NCORES = 8
H = B // NCORES  # heads per core
P = 128  # partitions
KT = S // P  # 16 k-tiles
QC = 512  # q-chunk
NQC = S // QC  # 4 q-chunks
NPAIR = H // 2  # head pairs per core

F32 = mybir.dt.float32
BF16 = mybir.dt.bfloat16
I16 = mybir.dt.int16
F16 = mybir.dt.float16

# Base number of k-tiles (of 16) whose exp runs on DVE via the Schraudolph
# bit trick; alternating chunks use DVE_BASE / DVE_BASE+1 tiles.
DVE_BASE = int(os.environ.get("BASS_ATTN_DVE_BASE", "6"))


def _dve_set(size):
    return {round((i + 0.5) * KT / size) % KT for i in range(size)}


# Schraudolph constants for bf16 exp via int16 bit pattern:
#   i = round_int16(x * 2^7/ln2 + b);  exp(x) ~= bitcast_bf16(i)
_SCH_A = float(128.0 / np.log(2.0))
_SCH_B = float(os.environ.get("BASS_ATTN_SCH_B", "16250.5"))


def build_attention_nc() -> bass.Bass:
    nc = bacc.Bacc()
    q_d = nc.declare_dram_parameter("q", [H, S, D], F32, isOutput=False)
    k_d = nc.declare_dram_parameter("k", [H, S, D], F32, isOutput=False)
    v_d = nc.declare_dram_parameter("v", [H, S, D], F32, isOutput=False)
    o_d = nc.declare_dram_parameter("out", [H, S, D], F32, isOutput=True)

    q_bp = q_d.rearrange("h (b i) (a j) -> h a i b j", i=32, j=32)
    k_bp = k_d.rearrange("h (b i) (a j) -> h a i b j", i=32, j=32)
    v_v = v_d.rearrange("h (t p) d -> h p t d", p=P)
    o_v = o_d.rearrange("h (g c p) d -> h p g c d", c=4, p=P)

    with tile.TileContext(nc) as tc:
        with (
            tc.tile_pool(name="consts", bufs=1) as consts,
            tc.tile_pool(name="stage", bufs=4) as stage,
            tc.tile_pool(name="qk_t", bufs=16) as qkt_pool,
            tc.tile_pool(name="vpool", bufs=4) as vpool,
            tc.tile_pool(name="ppool", bufs=6) as ppool,
            tc.tile_pool(name="osb", bufs=4) as osb_pool,
            tc.tile_pool(name="epsb", bufs=4) as epsb_pool,
            tc.tile_pool(name="outsb", bufs=4) as outsb_pool,
            tc.tile_pool(name="rz", bufs=4) as rz_pool,
            tc.tile_pool(name="spsum", bufs=3, space="PSUM") as spsum,
            tc.tile_pool(name="oep", bufs=2, space="PSUM") as opsum,
        ):
            ident = consts.tile([D + 1, D + 1], F32)
            make_identity(nc, ident[:])
            ones16 = consts.tile([P, KT], F32)
            nc.vector.memset(ones16[:], 1.0)

            # Deferred-work queue (prep of next pair + epilogue tails),
            # drained one closure per kt so per-engine FIFOs interleave
            # boundary work with steady-state work.
            pending = []

            def drain(n=1):
                for _ in range(min(n, len(pending))):
                    pending.pop(0)()

            def prep_closures(pair):
                h_a, h_b = 2 * pair, 2 * pair + 1
                qkt = {"q": [None] * 4, "k": [None] * 4}
                v_aug = {}
                out = []
                for fc in range(4):
                    for name, src in (("k", k_bp), ("q", q_bp)):

                        def go(fc=fc, name=name, src=src):
                            st = stage.tile([P, QC], F32, tag="stage")
                            for hh, pb in ((h_a, 0), (h_b, 2)):
                                for a in range(2):
                                    c = pb + a
                                    nc.sync.dma_start(
                                        out=st[32 * c : 32 * c + 32, :].rearrange(
                                            "i (b j) -> i b j", j=32
                                        ),
                                        in_=src[hh, a, :, 16 * fc : 16 * fc + 16, :],
                                    )
                            st2 = stage.tile([P, QC], F32, tag="stage2")
                            nc.vector.transpose(st2[:], st[:])
                            ch = qkt_pool.tile([P, QC], F16, tag="qkT")
                            nc.vector.tensor_copy(ch[:], st2[:])
                            qkt[name][fc] = ch

                        out.append(go)
                for hh, part in ((h_a, 0), (h_b, 1)):

                    def gov(hh=hh, part=part):
                        vst = stage.tile([P, KT, D], F32, tag="vstage")
                        nc.sync.dma_start(out=vst[:], in_=v_v[hh])
                        va = vpool.tile([P, KT, D + 1], BF16, tag="v")
                        nc.vector.tensor_copy(va[:, :, 0:D], vst[:])
                        nc.vector.tensor_copy(va[:, :, D], ones16[:])
                        v_aug[part] = va

                    out.append(gov)
                return out, qkt, v_aug

            def epilogue_core(o_ps):
                """Inline at next-chunk start (after its QK prefetch):
                PSUM->SBUF copies (ACT part0 / DVE part1), PE transposes,
                and ep->SBUF copies.  All PSUM readers are emitted here,
                so the 2-deep oep slot rotation (o_a, o_b, ep_a, ep_b per
                chunk) stays legal; the deferred tail is SBUF-only."""
                epcs = {}
                osbs = {}
                for part in (0, 1):
                    o_sb = osb_pool.tile([D + 1, QC], F32, tag="ot")
                    if part == 0:
                        nc.scalar.copy(o_sb[:], o_ps[part][:])
                    else:
                        nc.vector.tensor_copy(o_sb[:], o_ps[part][:])
                    osbs[part] = o_sb
                for part in (0, 1):
                    ep = opsum.tile([P, 4, D + 1], F32, tag="oep")
                    for c in range(4):
                        nc.tensor.transpose(
                            ep[:, c, :],
                            osbs[part][:, c * P : (c + 1) * P],
                            ident[:],
                        )
                    epc = epsb_pool.tile([P, 4, D + 1], F32, tag="eps")
                    if part == 0:
                        nc.scalar.copy(epc[:], ep[:])
                    else:
                        nc.vector.tensor_copy(epc[:], ep[:])
                    epcs[part] = epc
                return epcs

            def epilogue_tail_closures(pair, g, epcs):
                """Deferred into the next chunk, one per kt: 1/Z recip +
                scale on DVE -- all SBUF reads, so tensor_scalar gets the
                2-port mode -- then DMA out."""
                h_a, h_b = 2 * pair, 2 * pair + 1
                out = []
                for part, hh in ((0, h_a), (1, h_b)):

                    def e_fin(part=part, hh=hh):
                        epc = epcs[part]
                        rz = rz_pool.tile([P, 4], F32, tag="rz")
                        nc.vector.reciprocal(rz[:], epc[:, :, D])
                        out_sb = outsb_pool.tile([P, 4, D], F32, tag="out")
                        for c in range(4):
                            nc.vector.tensor_scalar(
                                out=out_sb[:, c, :],
                                in0=epc[:, c, 0:D],
                                scalar1=rz[:, c : c + 1],
                                scalar2=None,
                                op0=mybir.AluOpType.mult,
                            )
                        nc.sync.dma_start(out=o_v[hh, :, g, :, :], in_=out_sb[:])

                    out.append(e_fin)
                return out

            # startup: only the minimal prefix of pair-0 prep runs
            # eagerly (k-chunk 0, q-chunk 0, both V tiles); the remaining
            # six q/k chunk closures drain inside the first kt loop, each
            # provably before its first use (kT[i] first read when
            # emitting QK(4i) at kt 4i-3; qT[g] first read at chunk g).
            prep0, qkt, v_aug = prep_closures(0)
            eager = [prep0[0], prep0[1], prep0[8], prep0[9]]
            rest = prep0[2:8]
            for c in eager:
                c()
            pending.extend(rest)
            nxt = {}
            prev = None  # (pair, g, o_ps) of the chunk awaiting epilogue

            chunk_idx = 0
            for pair in range(NPAIR):
                if pair > 0:
                    qkt, v_aug = nxt[pair]
                    while any(t is None for t in qkt["q"] + qkt["k"]) or len(
                        v_aug
                    ) < 2:
                        pending.pop(0)()
                kT = qkt["k"]
                qT = qkt["q"]

                for g in range(NQC):

                    def emit_qkt(kt):
                        s_ps = spsum.tile([P, 2, QC], F32, tag="s")
                        k_ch = kT[kt // 4]
                        k_sl = slice((kt % 4) * P, (kt % 4 + 1) * P)
                        for part, base in ((0, 0), (1, 64)):
                            nc.tensor.matmul(
                                s_ps[:, part, :],
                                k_ch[base : base + 64, k_sl],
                                qT[g][base : base + 64, :],
                                tile_position=(base, 0),
                            )
                        return s_ps

                    # QK prefetch for this chunk FIRST, so the PE FIFO has
                    # work before the previous chunk's epilogue transposes.
                    s_tiles = {i: emit_qkt(i) for i in range(3)}

                    # previous chunk's epilogue core (frees its o_ps slots
                    # before we allocate ours from the 2-deep pool)
                    if prev is not None:
                        p_pair, p_g, p_ops = prev
                        eps = epilogue_core(p_ops)
                        pending.extend(epilogue_tail_closures(p_pair, p_g, eps))

                    o_ps_a = opsum.tile([D + 1, QC], F32, tag="oep")
                    o_ps_b = opsum.tile([D + 1, QC], F32, tag="oep")
                    o_ps = {0: o_ps_a, 1: o_ps_b}

                    dve_kts = _dve_set(DVE_BASE)
                    for kt in range(KT):
                        s_ps = s_tiles.pop(kt)
                        p_sb = ppool.tile([P, 2, QC], BF16, tag="p")
                        if kt in dve_kts:
                            nc.vector.tensor_scalar(
                                out=p_sb[:].bitcast(I16),
                                in0=s_ps[:],
                                scalar1=_SCH_A,
                                scalar2=_SCH_B,
                                op0=mybir.AluOpType.mult,
                                op1=mybir.AluOpType.add,
                            )
                        else:
                            nc.scalar.activation(
                                p_sb[:], s_ps[:], mybir.ActivationFunctionType.Exp
                            )
                        for part in (0, 1):
                            nc.tensor.matmul(
                                o_ps[part][:],
                                v_aug[part][:, kt, :],
                                p_sb[:, part, :],
                                start=(kt == 0),
                                stop=(kt == KT - 1),
                            )
                        if kt + 3 < KT:
                            s_tiles[kt + 3] = emit_qkt(kt + 3)
                        drain(1)

                    prev = (pair, g, o_ps)
                    if g == 1 and pair + 1 < NPAIR:
                        pc, qkt_n, v_n = prep_closures(pair + 1)
                        pending.extend(pc)
                        nxt[pair + 1] = (qkt_n, v_n)
                    chunk_idx += 1

            # final chunk's epilogue + leftover deferred work
            p_pair, p_g, p_ops = prev
            eps = epilogue_core(p_ops)
            pending.extend(epilogue_tail_closures(p_pair, p_g, eps))
            drain(len(pending))
    nc.finalize()
    return nc


_NC_CACHE = None


def _get_nc():
    global _NC_CACHE
    if _NC_CACHE is None:
        _NC_CACHE = build_attention_nc()
    return _NC_CACHE


def kernel(q: np.ndarray, k: np.ndarray, v: np.ndarray) -> np.ndarray:
    q = np.asarray(q, dtype=np.float32)
    k = np.asarray(k, dtype=np.float32)
    v = np.asarray(v, dtype=np.float32)
    nc = _get_nc()
    in_maps = [
        {
            "q": np.ascontiguousarray(q[c * H : (c + 1) * H]),
            "k": np.ascontiguousarray(k[c * H : (c + 1) * H]),
            "v": np.ascontiguousarray(v[c * H : (c + 1) * H]),
        }
        for c in range(NCORES)
    ]
    res = run_bass_kernel_spmd(nc, in_maps, list(range(NCORES)))
    return np.concatenate([res.results[c]["out"] for c in range(NCORES)], axis=0)
